# revision 44
# baseline (speedup 1.0000x reference)
"""Trainium2 Bass kernel for nn_Expansion (e3nn-style tensor-product expansion).

Math reformulation (verified against the jax reference):
  h   = silu(node_emb @ lw1 + lb1)                         [B,64]
  hb  = silu(node_emb @ bw1 + bb1)                         [B,64]
  x0  = feat[:,:128] @ W0 / sqrt(128)                      [B,16]
  x1k = feat[:,128+k::3] @ W1 / 8          (k=0,1,2)       [B,16]

The per-sample path contractions with w_path = (h @ lw2 + lb2) sliced are a
batched bilinear form

  r[b,p] = sum_{c,w} h'[b,c] x[b,w] M[(c,w), p],   h' = [h, 1]

i.e. a plain matmul over the outer product  z[b,(c,w)] = h'[b,c]*x[b,w]
(K = 65*16 = 1040; the c=64 block is lb2 and is skipped when lb2 == 0)
against reshaped weight matrices M built from lw2/lb2 on the host.  This
avoids materializing w = h@lw2 ([B,36864], ~600 MB) entirely.

Sharding: pure data parallel, batch 4096 -> 8 cores x 512.  Weights replicated.

Device layout per core (B_c = 512):
  - All activations load as bf16 with the contraction dim on partitions.
  - The partition-replicated tiles the z outer product needs are produced
    DIRECTLY by the pre-matmuls: host-side column-replicated weights
    (LW1R = lw1[:, repeat], W0R/W1R = tile(W,8)) make the PE emit
    hbc[q][p,b] = h_pre[8q+p//16, b] and xbc[t][p,b] = x_t[p%16, b]; the
    SiLU (with replicated per-partition bias) and PSUM->SBUF casts land
    them in SBUF as bf16.
  - z tiles (DVE bf16 multiplies) feed the main matmuls: out[128b, N<=512]
    accumulated over 8 K-chunks (+ 65-row bias-MLP chunk for blk00/blk11).
  - A burst of dummy warm-up matmuls at t=0 ramps the PE clock out of its
    cold p-state before the real work arrives.
  - Big weight matrices stream via the Pool/SWDGE DMA path so the critical
    activation loads own the HWDGE queue; output writes are split into
    1280-column chunks so the writeback pipelines tightly with compute.
All path normalization constants are folded into the host-side weight prep.
"""

import sys

import numpy as np

sys.path.insert(0, "/opt/trn_rl_repo")

import ml_dtypes  # noqa: E402

B_TOTAL = 4096
N_CORES = 8
BC = B_TOTAL // N_CORES  # 512 samples per core
P = 128
NB = BC // P  # 4 b-tiles per core
C3 = 1.0 / np.sqrt(3.0)

MM_MODE = "bf16"
N_WARM = 10

_CACHE = {}


def _build_program(mode, skip_lb2):
    import concourse.tile as tile
    from concourse import bacc, mybir

    F32 = mybir.dt.float32
    MM = mybir.dt.bfloat16
    AF = mybir.ActivationFunctionType

    nc = bacc.Bacc("TRN2", target_bir_lowering=False, debug=False,
                   num_devices=N_CORES)

    t = {}
    # blob16: [emb 512 | lw1r_q0 128 | lb1r 8 | bb1 1 || feats 512 |
    #          W0R 128 | lw1r_q1:8 896 || W1R(pad) 128 | BW1 64]
    # -> [128, 2377] bf16; the first 649 columns alone unlock the hbc0
    # matmul + SiLU, the next 640 the xbc0 matmul
    t["blob16"] = nc.dram_tensor("blob16", [P, 2377], MM, kind="ExternalInput").ap()
    t["featv"] = nc.dram_tensor("featv", [64, 3, BC], MM, kind="ExternalInput").ap()
    t["R0"] = nc.dram_tensor("R0", [1040, 1280], MM, kind="ExternalInput").ap()
    t["R1"] = nc.dram_tensor("R1", [1040, 1024], MM, kind="ExternalInput").ap()
    t["BB"] = nc.dram_tensor("BB", [65, 1280], MM, kind="ExternalInput").ap()
    t["out"] = nc.dram_tensor("out", [BC, 6400], F32, kind="ExternalOutput").ap()

    with tile.TileContext(nc) as tc:
        _emit(tc, t, skip_lb2, mybir, MM, F32, AF)

    nc.compile()
    return nc


def _emit(tc, t, skip_lb2, mybir, MM, F32, AF):
    nc = tc.nc
    from contextlib import ExitStack

    with ExitStack() as ctx:
        wpool = ctx.enter_context(tc.tile_pool(name="weights", bufs=1))
        apool = ctx.enter_context(tc.tile_pool(name="acts", bufs=1))
        zpool = ctx.enter_context(tc.tile_pool(name="z", bufs=1))
        opool = ctx.enter_context(tc.tile_pool(name="outs", bufs=3))
        prex_psum = ctx.enter_context(tc.tile_pool(name="prex_psum", bufs=4, space="PSUM"))
        main_psum = ctx.enter_context(tc.tile_pool(name="main_psum", bufs=4, space="PSUM"))

        # ---- PE warm-up: ramp the clock out of the cold p-state while the
        #      input DMAs are still in flight ----
        wsrc = wpool.tile([P, 256], MM, tag="wsrc")
        nc.vector.memset(wsrc[:], 0.0)
        wp = prex_psum.tile([64, 256], F32, name="warm", tag="px")
        for _ in range(N_WARM):
            nc.tensor.matmul(wp[:], lhsT=wsrc[:, 0:64], rhs=wsrc[:],
                             start=True, stop=True)

        # ---- weights / inputs to SBUF ----
        # critical-path activation loads in one packed DMA each (HWDGE
        # descriptor-gen costs ~0.6us per DMA, so fewer+bigger wins); the
        # big matmul weights stream via Pool/SWDGE off the HWDGE queue
        # three separate SBUF tiles so tile-granularity dependency tracking
        # doesn't make early consumers wait on later chunks
        blobA = wpool.tile([P, 649], MM, tag="blobA")
        blobB = wpool.tile([P, 1536], MM, tag="blobB")
        blobC = wpool.tile([P, 192], MM, tag="blobC")
        featv_sb = apool.tile([64, 3, BC], MM, tag="featv")
        BB_sb = wpool.tile([65, 1280], MM, tag="BB")
        R0_sb = wpool.tile([P, 9, 1280], MM, tag="R0")
        R1_sb = wpool.tile([P, 9, 1024], MM, tag="R1")

        emb_sb = blobA[:, 0:512]
        lb1r_sb = blobA[:, 640:648]
        bb1_sb = blobA[0:64, 648:649]
        feats_sb = blobB[:, 0:512]
        w0r_sb = blobB[:, 512:640]
        w1r_sb = blobC[0:64, 0:128]
        bw1_sb = blobC[:, 128:192]

        def lw1r_q(q):
            # q0 rides in the first blob chunk; q1..7 in the second
            if q == 0:
                return blobA[:, 512:640]
            return blobB[:, 640 + P * (q - 1):640 + P * q]

        nc.sync.dma_start(blobA[:], t["blob16"][:, 0:649])
        nc.sync.dma_start(blobB[:], t["blob16"][:, 649:2185])
        nc.sync.dma_start(blobC[:], t["blob16"][:, 2185:2377])
        nc.sync.dma_start(featv_sb[:], t["featv"][:])
        nc.sync.dma_start(BB_sb[:], t["BB"][:])

        r0v = t["R0"][0:1024].rearrange("(q p) n -> p q n", p=P)
        r1v = t["R1"][0:1024].rearrange("(q p) n -> p q n", p=P)
        # delay the SWDGE prefetch just long enough that the critical
        # activation loads win the DMA device, then stream the R chunks in
        # main-bank consumption order (p00a, p01*, p00b, p11, p10*);
        # R1c0/R0c1 go in q-halves so their banks start on the early half
        dly = wpool.tile([P, 780], MM, tag="dly")
        nc.gpsimd.memset(dly[:], 0.0)
        nc.gpsimd.dma_start(R0_sb[:, 0:8, 0:512], r0v[:, :, 0:512])
        nc.gpsimd.dma_start(R1_sb[:, 0:4, 0:512], r1v[:, 0:4, 0:512])
        nc.gpsimd.dma_start(R1_sb[:, 4:8, 0:512], r1v[:, 4:8, 0:512])
        nc.gpsimd.dma_start(R0_sb[:, 0:4, 512:1024], r0v[:, 0:4, 512:1024])
        nc.gpsimd.dma_start(R0_sb[:, 4:8, 512:1024], r0v[:, 4:8, 512:1024])
        nc.gpsimd.dma_start(R1_sb[:, 0:8, 512:1024], r1v[:, :, 512:1024])
        nc.gpsimd.dma_start(R0_sb[:, 0:8, 1024:1280], r0v[:, :, 1024:1280])
        if not skip_lb2:
            nc.sync.dma_start(R0_sb[0:16, 8, :], t["R0"][1024:1040])
            nc.sync.dma_start(R1_sb[0:16, 8, :], t["R1"][1024:1040])

        # ---- prep: replicated h (SiLU) and x tiles straight off the PE ----
        hbc = [apool.tile([P, BC], MM, name=f"hbc{q}", tag=f"hbc{q}")
               for q in range(8)]
        xbc = [apool.tile([P, BC], MM, name=f"xbc{t_}", tag=f"xbc{t_}")
               for t_ in range(4)]
        hbp_sb = apool.tile([65, BC], MM, tag="hbp")

        def hbc_mm(q):
            ph = prex_psum.tile([P, BC], F32, name=f"phbc{q}", tag="px")
            nc.tensor.matmul(ph[:], lhsT=lw1r_q(q),
                             rhs=emb_sb[:], start=True, stop=True)
            nc.scalar.activation(hbc[q][:], ph[:], AF.Silu,
                                 bias=lb1r_sb[:, q:q + 1])

        def xbc_mm(tdx):
            px = prex_psum.tile([P, BC], F32, name=f"pxbc{tdx}", tag="px")
            if tdx == 0:
                nc.tensor.matmul(px[:], lhsT=w0r_sb[:], rhs=feats_sb[:],
                                 start=True, stop=True)
            else:
                nc.tensor.matmul(px[:], lhsT=w1r_sb[:],
                                 rhs=featv_sb[:, tdx - 1, :],
                                 start=True, stop=True)
            nc.vector.tensor_copy(out=xbc[tdx][:], in_=px[:])

        # hbc0 first (its operands arrive in the first blob chunk), then
        # xbc0; the featv-dependent xbc1..3 go AFTER the whole h chain so a
        # late featv can't block the in-order PE queue
        hbc_mm(0)
        xbc_mm(0)
        for q in range(1, 8):
            hbc_mm(q)

        # bias-MLP head hb' = [silu(emb@bw1+bb1), 1]
        pb = prex_psum.tile([64, BC], F32, name="phbp", tag="px")
        nc.tensor.matmul(pb[:], lhsT=bw1_sb[:], rhs=emb_sb[:],
                         start=True, stop=True)
        nc.scalar.activation(hbp_sb[0:64, :], pb[:], AF.Silu, bias=bb1_sb[:])
        nc.vector.memset(hbp_sb[64:65, :], 1.0)

        xbc_mm(1)
        xbc_mm(2)
        xbc_mm(3)

        # ---- z outer-product tiles (DVE bf16) ----
        # Sliced per b-tile (the main matmuls for tile j only read columns
        # [128j:128j+128]) and produced one tile ahead, so tile-0 banks
        # start ~4us earlier and later tiles never wait on DVE.
        z = [[zpool.tile([P, BC], MM, name=f"z{tdx}_{q}", tag=f"z{tdx}_{q}")
              for q in range(8)] for tdx in range(4)]

        def z_slice(j):
            bsl = slice(P * j, P * (j + 1))
            for tdx in range(4):
                for q in range(8):
                    nc.vector.tensor_mul(out=z[tdx][q][:, bsl],
                                         in0=hbc[q][:, bsl],
                                         in1=xbc[tdx][:, bsl])

        z_slice(0)

        # ---- main matmuls + output assembly ----
        def accum2(tdx, rhs_sb, col0, ncols, bias_cols, bsl, psum_ap):
            # bias and lb2 terms FIRST so the accumulation group (and with it
            # the PSUM->SBUF copy) closes on the last z chunk, not on a
            # trailing extra matmul
            nmm = 8 + (0 if skip_lb2 else 1) + (1 if bias_cols is not None else 0)
            idx = 0
            if bias_cols is not None:
                idx += 1
                nc.tensor.matmul(psum_ap,
                                 lhsT=hbp_sb[:, bsl],
                                 rhs=BB_sb[:, bias_cols[0]:bias_cols[1]],
                                 start=True, stop=False)
            if not skip_lb2:
                idx += 1
                nc.tensor.matmul(psum_ap,
                                 lhsT=xbc[tdx][0:16, bsl],
                                 rhs=rhs_sb[0:16, 8, col0:col0 + ncols],
                                 start=(idx == 1), stop=False)
            for q in range(8):
                idx += 1
                nc.tensor.matmul(psum_ap,
                                 lhsT=z[tdx][q][:, bsl],
                                 rhs=rhs_sb[:, q, col0:col0 + ncols],
                                 start=(idx == 1), stop=(idx == nmm))

        def emit_p01(j, out_t, o3, top, bsl):
            # r01k -> blk01: out[u, 32+3v+k], u<32, v<16
            for k in range(3):
                p01 = main_psum.tile([P, 512], F32, name=f"p01_{k}", tag="mp")
                accum2(1 + k, R1_sb, 0, 512, None, bsl, p01[:])
                dst = top[:, :, 32:80].rearrange(
                    "p u (v jj) -> p u v jj", jj=3)[:, :, :, k]      # [128,32,16]
                src = p01[:].rearrange("p (u v) -> p u v", v=16)
                if k == 0:
                    nc.scalar.copy(dst, src)
                else:
                    nc.vector.tensor_copy(out=dst, in_=src)

        def emit_p00(j, out_t, o3, bsl, which, fine_tail):
            # r00 -> blk00 rows 0..15 (a) / 16..31 (b), plus the top write
            # for those rows once the p01 columns are in place
            col0, bias, r0, wa, wb = (
                (0, (0, 512), 0, 0, 1280) if which == 0
                else (512, (512, 1024), 16, 1280, 2560))
            p00 = main_psum.tile([P, 512], F32, name=f"p00{which}", tag="mp")
            accum2(0, R0_sb, col0, 512, bias, bsl, p00[:])
            pv = p00[:].rearrange("p (u v) -> p u v", v=32)
            if fine_tail:
                # the tail writebacks: copy + DMA in row-halves so the
                # final transfers are 640 columns each
                wm = (wa + wb) // 2
                nc.scalar.copy(o3[:, r0:r0 + 8, 0:32], pv[:, 0:8])
                nc.sync.dma_start(t["out"][bsl, wa:wm], out_t[:, wa:wm])
                nc.scalar.copy(o3[:, r0 + 8:r0 + 16, 0:32], pv[:, 8:16])
                nc.sync.dma_start(t["out"][bsl, wm:wb], out_t[:, wm:wb])
            else:
                nc.scalar.copy(o3[:, r0:r0 + 16, 0:32], pv)
                nc.sync.dma_start(t["out"][bsl, wa:wb], out_t[:, wa:wb])

        def emit_top(j, out_t, o3, top, bsl, fine_tail=False):
            if fine_tail:
                # last tile: p01 banks first so the row-0:16 write overlaps
                # the final p00 banks
                emit_p01(j, out_t, o3, top, bsl)
                emit_p00(j, out_t, o3, bsl, 0, True)
                emit_p00(j, out_t, o3, bsl, 1, True)
            else:
                emit_p00a_then_p01(j, out_t, o3, top, bsl)

        def emit_p00a_then_p01(j, out_t, o3, top, bsl):
            p00a = main_psum.tile([P, 512], F32, name="p00a", tag="mp")
            accum2(0, R0_sb, 0, 512, (0, 512), bsl, p00a[:])
            nc.scalar.copy(o3[:, 0:16, 0:32],
                           p00a[:].rearrange("p (u v) -> p u v", v=32))
            emit_p01(j, out_t, o3, top, bsl)
            nc.sync.dma_start(t["out"][bsl, 0:1280], out_t[:, 0:1280])
            p00b = main_psum.tile([P, 512], F32, name="p00b", tag="mp")
            accum2(0, R0_sb, 512, 512, (512, 1024), bsl, p00b[:])
            nc.scalar.copy(o3[:, 16:32, 0:32],
                           p00b[:].rearrange("p (u v) -> p u v", v=32))
            nc.sync.dma_start(t["out"][bsl, 1280:2560], out_t[:, 1280:2560])

        def emit_bot(j, out_t, o3, bot, bsl):
            # r10i -> blk10: out[32+3u+i, v], u<16, v<32
            # (before r11 so the bank order matches R-chunk arrival order)
            for i in range(3):
                p10 = main_psum.tile([P, 512], F32, name=f"p10_{i}", tag="mp")
                accum2(1 + i, R1_sb, 512, 512, None, bsl, p10[:])
                dst = bot[:, :, i, 0:32]                             # [128,16,32]
                src = p10[:].rearrange("p (u v) -> p u v", v=32)
                if i == 1:
                    nc.vector.tensor_copy(out=dst, in_=src)
                else:
                    # i==0,2 on ACT: GPSIMD cannot read PSUM
                    nc.scalar.copy(dst, src)

            # r11 -> blk11 diagonal-in-(i,j): out[32+3u+i, 32+3v+i]
            p11 = main_psum.tile([P, 512], F32, name="p11", tag="mp")
            accum2(0, R0_sb, 1024, 256, (1024, 1280), bsl, p11[:, 0:256])
            src11 = p11[:, 0:256].rearrange("p (u v) -> p u v", v=16)
            for i in range(3):
                dst = bot[:, :, i, 32:80].rearrange(
                    "p u (v jj) -> p u v jj", jj=3)[:, :, :, i]      # [128,16,16]
                if i == 1:
                    nc.vector.tensor_copy(out=dst, in_=src11)
                else:
                    # i==0,2 on ACT: GPSIMD cannot read PSUM
                    nc.scalar.copy(dst, src11)

            # bottom half in three chunks for tight writeback pipelining
            nc.sync.dma_start(t["out"][bsl, 2560:3840], out_t[:, 2560:3840])
            nc.sync.dma_start(t["out"][bsl, 3840:5120], out_t[:, 3840:5120])
            nc.sync.dma_start(t["out"][bsl, 5120:6400], out_t[:, 5120:6400])

        for j in range(NB):
            bsl = slice(P * j, P * (j + 1))
            out_t = opool.tile([P, 6400], F32, name="out_t", tag="out_t")
            o3 = out_t.rearrange("p (r c) -> p r c", c=80)          # [128,80,80]
            top = o3[:, 0:32, :]                                     # [128,32,80]
            bot = out_t[:, 2560:6400].rearrange(
                "p (u i c) -> p u i c", i=3, c=80)                   # [128,16,3,80]

            # blk11 off-diagonal zeros
            nc.gpsimd.memset(o3[:, 32:80, 32:80], 0.0)

            if j < NB - 1:
                emit_top(j, out_t, o3, top, bsl)
                # next tile's z slices build on DVE under this tile's
                # bottom banks (after the top copies, which gate the
                # first writeback)
                z_slice(j + 1)
                emit_bot(j, out_t, o3, bot, bsl)
            else:
                # last tile bottom-first: the tail then ends on the two
                # small top writes instead of the three bottom ones
                emit_bot(j, out_t, o3, bot, bsl)
                emit_top(j, out_t, o3, top, bsl, fine_tail=True)


def _prepare(inputs, mode):
    f32 = np.float32
    bf16 = ml_dtypes.bfloat16
    feat = np.ascontiguousarray(np.asarray(inputs["feat"], dtype=f32))
    node_emb = np.ascontiguousarray(np.asarray(inputs["node_emb"], dtype=f32))
    W0 = np.asarray(inputs["W0"], f32)
    W1 = np.asarray(inputs["W1"], f32)
    lw1 = np.asarray(inputs["lw1"], f32)
    lb1 = np.asarray(inputs["lb1"], f32)
    lw2 = np.asarray(inputs["lw2"], f32)
    lb2 = np.asarray(inputs["lb2"], f32)
    bw1 = np.asarray(inputs["bw1"], f32)
    bb1 = np.asarray(inputs["bb1"], f32)
    bw2 = np.asarray(inputs["bw2"], f32)
    bb2 = np.asarray(inputs["bb2"], f32)

    s16 = np.float32(1.0 / 16.0)
    sC = np.float32(C3 / 16.0)

    lw2p = np.concatenate([lw2, lb2[None]], axis=0)           # [65, 36864]
    M00 = lw2p[:, :16384].reshape(1040, 1024) * s16
    M11 = lw2p[:, 16384:20480].reshape(1040, 256) * sC
    M01 = lw2p[:, 20480:28672].reshape(1040, 512) * sC
    M10 = lw2p[:, 28672:36864].reshape(1040, 512) * sC
    R0 = np.ascontiguousarray(np.concatenate([M00, M11], axis=1)).astype(bf16)
    R1 = np.ascontiguousarray(np.concatenate([M01, M10], axis=1)).astype(bf16)
    BBf = np.concatenate([bw2, bb2[None]], axis=0)            # [65, 1280]
    BB = np.ascontiguousarray(
        np.concatenate([BBf[:, :1024] * s16, BBf[:, 1024:] * sC], axis=1)
    ).astype(bf16)

    W0s = W0 * np.float32(1.0 / np.sqrt(128.0))
    W1s = W1 * np.float32(1.0 / 8.0)

    # column-replicated pre-matmul weights: the PE emits partition-replicated
    # activations directly (see module docstring)
    rep = np.repeat(np.arange(64), 16)                        # c = j // 16
    LW1R = lw1[:, rep].astype(bf16)                           # [128, 1024]
    LB1R = np.empty((P, 8), f32)
    for q in range(8):
        LB1R[:, q] = lb1[8 * q + np.arange(P) // 16]
    W0R = np.tile(W0s, (1, 8)).astype(bf16)                   # [128, 128]
    W1R = np.zeros((P, P), bf16)
    W1R[0:64] = np.tile(W1s, (1, 8)).astype(bf16)             # [64, 128] + pad
    BW1 = bw1.astype(bf16)

    LB1Rb = LB1R.astype(bf16)                                 # [128, 8]
    bb1b = np.zeros((P, 1), bf16)
    bb1b[0:64, 0] = bb1.astype(bf16)

    skip_lb2 = not bool(np.any(lb2))

    in_maps = []
    for i in range(N_CORES):
        sl = slice(i * BC, (i + 1) * BC)
        fs = feat[sl]
        embT = node_emb[sl].T.astype(bf16)                    # [128, BC]
        featsT = fs[:, :128].T.astype(bf16)                   # [128, BC]
        featv = np.stack(
            [fs[:, 128 + k::3].T.astype(bf16) for k in range(3)], axis=1
        )                                                     # [64, 3, BC]
        blob16 = np.ascontiguousarray(np.concatenate(
            [embT, LW1R[:, 0:128], LB1Rb, bb1b, featsT, W0R,
             LW1R[:, 128:1024], W1R, BW1], axis=1))           # [128, 2377]
        in_maps.append({
            "blob16": blob16,
            "featv": np.ascontiguousarray(featv),
            "R0": R0, "R1": R1, "BB": BB,
        })
    return in_maps, skip_lb2


def run(inputs, mode=None, trace=False):
    """Build (cached), run on 8 cores, gather. Returns (out, results)."""
    mode = mode or MM_MODE
    in_maps, skip_lb2 = _prepare(inputs, mode)
    key = (mode, skip_lb2)
    if key not in _CACHE:
        _CACHE[key] = _build_program(mode, skip_lb2)
    nc = _CACHE[key]

    from concourse.bass_utils import run_bass_kernel_spmd
    res = run_bass_kernel_spmd(nc, in_maps, list(range(N_CORES)), trace=trace)
    out = np.concatenate(
        [res.results[i]["out"].reshape(BC, 80, 80) for i in range(N_CORES)],
        axis=0)
    return out.astype(np.float32), res


def kernel(**inputs):
    out, _ = run(inputs)
    return out


# revision 80
# speedup vs baseline: 1.0267x; 1.0267x over previous
"""Trainium2 Bass kernel for nn_Expansion (e3nn-style tensor-product expansion).

Math reformulation (verified against the jax reference):
  h   = silu(node_emb @ lw1 + lb1)                         [B,64]
  hb  = silu(node_emb @ bw1 + bb1)                         [B,64]
  x0  = feat[:,:128] @ W0 / sqrt(128)                      [B,16]
  x1k = feat[:,128+k::3] @ W1 / 8          (k=0,1,2)       [B,16]

The per-sample path contractions with w_path = (h @ lw2 + lb2) sliced are a
batched bilinear form

  r[b,p] = sum_{c,w} h'[b,c] x[b,w] M[(c,w), p],   h' = [h, 1]

i.e. a plain matmul over the outer product  z[b,(c,w)] = h'[b,c]*x[b,w]
(K = 65*16 = 1040; the c=64 block is lb2 and is skipped when lb2 == 0)
against reshaped weight matrices M built from lw2/lb2 on the host.  This
avoids materializing w = h@lw2 ([B,36864], ~600 MB) entirely.

Sharding: pure data parallel, batch 4096 -> 8 cores x 512.  Weights replicated.

Device layout per core (B_c = 512):
  - All activations load as bf16 with the contraction dim on partitions.
  - h and hb come from ONE stacked [lw1|bw1] matmul + ONE SiLU (cost is per
    free element, so the second head is free; bb2==0 removes the ones row,
    and the BB bias weights sit at partitions 64:128 to align with hb).
  - h is then partition-replicated POST-activation by one-hot Gsel matmuls
    (hbc[q][p,b] = h[8q+p//16, b]); their PSUM->SBUF copies split across
    ACT (q0,2,4,5,6) and DVE (q1,3,7) -- the two engines able to read PSUM
    -- which is the prep phase's bandwidth limit.  xbc[t][p,b] = x_t[p%16,b]
    comes straight from host-tiled W0R/W1R matmuls.
  - z tiles (DVE bf16 multiplies, sliced per b-tile and built one tile
    ahead) feed the main matmuls: out[128b, N<=512] accumulated over 8
    K-chunks + a K=64 hb@BB bias chunk for blk00/blk11.
  - A dummy-activation pre-fires the ~1.3us SiLU table load and a burst of
    warm-up matmuls ramps the PE clock before real work arrives.
  - Critical loads arrive as packed blob DMAs ordered by consumption; the
    Gsel block and big R matrices stream via Pool/SWDGE off the HWDGE
    queue, R in main-bank consumption order with q-halved early chunks.
  - Output writes stream per assembled region; the last tile runs
    bottom-first and quarters its p00 banks so the final writebacks are
    640/320-column transfers right behind the closing matmuls.
All path normalization constants are folded into the host-side weight prep.
"""

import sys

import numpy as np

sys.path.insert(0, "/opt/trn_rl_repo")

import ml_dtypes  # noqa: E402

B_TOTAL = 4096
N_CORES = 8
BC = B_TOTAL // N_CORES  # 512 samples per core
P = 128
NB = BC // P  # 4 b-tiles per core
C3 = 1.0 / np.sqrt(3.0)

MM_MODE = "bf16"
N_WARM = 10

_CACHE = {}


def _build_program(mode, skip_lb2, skip_bb2):
    import concourse.tile as tile
    from concourse import bacc, mybir

    F32 = mybir.dt.float32
    MM = mybir.dt.bfloat16
    AF = mybir.ActivationFunctionType

    nc = bacc.Bacc("TRN2", target_bir_lowering=False, debug=False,
                   num_devices=N_CORES)

    t = {}
    # blob16: [emb 512 | lw1|bw1 128 | lb1|bb1(stacked) 1 || feats 512 |
    #          W0R 128 | Gsel(pad) 1024 | W1R(pad) 128]
    # -> [128, 2433] bf16; the first 641 columns alone unlock the combined
    # h/hb pre-activation + single SiLU (bb2==0 so no ones-row needed)
    t["blob16"] = nc.dram_tensor("blob16", [P, 2433], MM, kind="ExternalInput").ap()
    t["featv"] = nc.dram_tensor("featv", [64, 3, BC], MM, kind="ExternalInput").ap()
    t["R0"] = nc.dram_tensor("R0", [1040, 1280], MM, kind="ExternalInput").ap()
    t["R1"] = nc.dram_tensor("R1", [1040, 1024], MM, kind="ExternalInput").ap()
    t["BB"] = nc.dram_tensor("BB", [64, 1280], MM, kind="ExternalInput").ap()
    if not skip_bb2:
        t["BB2"] = nc.dram_tensor("BB2", [1, 1280], MM, kind="ExternalInput").ap()
    t["out"] = nc.dram_tensor("out", [BC, 6400], F32, kind="ExternalOutput").ap()

    with tile.TileContext(nc) as tc:
        _emit(tc, t, skip_lb2, skip_bb2, mybir, MM, F32, AF)

    nc.compile()
    return nc


def _emit(tc, t, skip_lb2, skip_bb2, mybir, MM, F32, AF):
    nc = tc.nc
    from contextlib import ExitStack

    with ExitStack() as ctx:
        wpool = ctx.enter_context(tc.tile_pool(name="weights", bufs=1))
        apool = ctx.enter_context(tc.tile_pool(name="acts", bufs=1))
        zpool = ctx.enter_context(tc.tile_pool(name="z", bufs=1))
        opool = ctx.enter_context(tc.tile_pool(name="outs", bufs=3))
        prex_psum = ctx.enter_context(tc.tile_pool(name="prex_psum", bufs=5, space="PSUM"))
        main_psum = ctx.enter_context(tc.tile_pool(name="main_psum", bufs=3, space="PSUM"))

        # ---- PE warm-up: ramp the clock out of the cold p-state while the
        #      input DMAs are still in flight ----
        wsrc = wpool.tile([P, 256], MM, tag="wsrc")
        nc.vector.memset(wsrc[:], 0.0)
        # dummy activation pre-fires the ~1.3us SiLU table load so it is
        # off the h-chain critical path
        wact = wpool.tile([64, 64], MM, tag="wact")
        nc.scalar.activation(wact[:], wsrc[0:64, 0:64], AF.Silu)
        wp = prex_psum.tile([64, 256], F32, name="warm", tag="px")
        for _ in range(N_WARM):
            nc.tensor.matmul(wp[:], lhsT=wsrc[:, 0:64], rhs=wsrc[:],
                             start=True, stop=True)

        # ---- weights / inputs to SBUF ----
        # critical-path activation loads in one packed DMA each (HWDGE
        # descriptor-gen costs ~0.6us per DMA, so fewer+bigger wins); the
        # big matmul weights stream via Pool/SWDGE off the HWDGE queue
        # three separate SBUF tiles so tile-granularity dependency tracking
        # doesn't make early consumers wait on later chunks
        blobA = wpool.tile([P, 641], MM, tag="blobA")
        blobB = wpool.tile([P, 640], MM, tag="blobB")
        blobG = wpool.tile([P, 1152], MM, tag="blobG")
        featv_sb = apool.tile([64, 3, BC], MM, tag="featv")
        # BB lives at partitions 64:128 to align with hb's rows in hh_sb
        BBt = wpool.tile([P, 1280], MM, tag="BB")
        BB_sb = BBt[64:128, :]
        if not skip_bb2:
            BB2_sb = wpool.tile([1, 1280], MM, tag="BB2")
            ones_sb = wpool.tile([1, BC], MM, tag="ones")
            nc.vector.memset(ones_sb[:], 1.0)
        R0_sb = wpool.tile([P, 9, 1280], MM, tag="R0")
        R1_sb = wpool.tile([P, 9, 1024], MM, tag="R1")

        emb_sb = blobA[:, 0:512]
        lwb1_sb = blobA[:, 512:640]
        lbb1_sb = blobA[:, 640:641]
        feats_sb = blobB[:, 0:512]
        w0r_sb = blobB[:, 512:640]
        gsel_sb = blobG[0:64, 0:1024]
        w1r_sb = blobG[0:64, 1024:1152]

        nc.sync.dma_start(blobA[:], t["blob16"][:, 0:641])
        nc.sync.dma_start(blobB[:], t["blob16"][:, 641:1281])
        nc.sync.dma_start(featv_sb[:], t["featv"][:])
        nc.sync.dma_start(BB_sb[:], t["BB"][:])
        if not skip_bb2:
            nc.sync.dma_start(BB2_sb[:], t["BB2"][:])

        r0v = t["R0"][0:1024].rearrange("(q p) n -> p q n", p=P)
        r1v = t["R1"][0:1024].rearrange("(q p) n -> p q n", p=P)
        # delay the SWDGE prefetch just long enough that the critical
        # activation loads win the DMA device, then stream the R chunks in
        # main-bank consumption order (p00a, p01*, p00b, p11, p10*);
        # R1c0/R0c1 go in q-halves so their banks start on the early half
        dly = wpool.tile([P, 600], MM, tag="dly")
        nc.gpsimd.memset(dly[:], 0.0)
        nc.gpsimd.dma_start(R0_sb[:, 0:8, 0:512], r0v[:, :, 0:512])
        nc.gpsimd.dma_start(R1_sb[:, 0:4, 0:512], r1v[:, 0:4, 0:512])
        nc.gpsimd.dma_start(R1_sb[:, 4:8, 0:512], r1v[:, 4:8, 0:512])
        nc.gpsimd.dma_start(R0_sb[:, 0:4, 512:1024], r0v[:, 0:4, 512:1024])
        nc.gpsimd.dma_start(R0_sb[:, 4:8, 512:1024], r0v[:, 4:8, 512:1024])
        nc.gpsimd.dma_start(R1_sb[:, 0:8, 512:1024], r1v[:, :, 512:1024])
        nc.gpsimd.dma_start(R0_sb[:, 0:8, 1024:1280], r0v[:, :, 1024:1280])
        if not skip_lb2:
            nc.sync.dma_start(R0_sb[0:16, 8, :], t["R0"][1024:1040])
            nc.sync.dma_start(R1_sb[0:16, 8, :], t["R1"][1024:1040])

        # ---- prep: replicated h (SiLU) and x tiles straight off the PE ----
        hbc = [apool.tile([P, BC], MM, name=f"hbc{q}", tag=f"hbc{q}")
               for q in range(8)]
        xbc = [apool.tile([P, BC], MM, name=f"xbc{t_}", tag=f"xbc{t_}")
               for t_ in range(4)]

        # h and hb computed in ONE stacked matmul + ONE SiLU (bb2 == 0, so
        # the bias-MLP needs no ones row); h is then replicated
        # post-activation by one-hot PE matmuls whose PSUM->SBUF copies
        # spread over ACT+DVE
        hh_sb = apool.tile([P, BC], MM, tag="hh")
        hp_sb = hh_sb[0:64, :]

        def h_mm():
            ph = prex_psum.tile([P, BC], F32, name="ph", tag="px")
            nc.tensor.matmul(ph[:], lhsT=lwb1_sb, rhs=emb_sb[:],
                             start=True, stop=True)
            nc.scalar.activation(hh_sb[:], ph[:], AF.Silu, bias=lbb1_sb)

        def hbc_mm(q):
            ph = prex_psum.tile([P, BC], F32, name=f"phbc{q}", tag="px")
            nc.tensor.matmul(ph[:], lhsT=gsel_sb[:, P * q:P * (q + 1)],
                             rhs=hp_sb[:], start=True, stop=True)
            if q in (1, 3, 7):
                nc.vector.tensor_copy(out=hbc[q][:], in_=ph[:])
            else:
                nc.scalar.copy(hbc[q][:], ph[:])

        def xbc_mm(tdx):
            px = prex_psum.tile([P, BC], F32, name=f"pxbc{tdx}", tag="px")
            if tdx == 0:
                nc.tensor.matmul(px[:], lhsT=w0r_sb[:], rhs=feats_sb[:],
                                 start=True, stop=True)
            else:
                nc.tensor.matmul(px[:], lhsT=w1r_sb[:],
                                 rhs=featv_sb[:, tdx - 1, :],
                                 start=True, stop=True)
            nc.vector.tensor_copy(out=xbc[tdx][:], in_=px[:])

        # h first (its operands arrive in the first blob chunk), then
        # xbc0; the featv-dependent xbc1..3 go AFTER the whole h chain so a
        # late featv can't block the in-order PE queue
        h_mm()
        xbc_mm(0)
        for q in range(8):
            hbc_mm(q)




        # ---- z outer-product tiles (DVE bf16) ----
        # Sliced per b-tile (the main matmuls for tile j only read columns
        # [128j:128j+128]) and produced one tile ahead, so tile-0 banks
        # start ~4us earlier and later tiles never wait on DVE.
        z = [[zpool.tile([P, BC], MM, name=f"z{tdx}_{q}", tag=f"z{tdx}_{q}")
              for q in range(8)] for tdx in range(4)]

        def z_slice_t(j, tdx):
            bsl = slice(P * j, P * (j + 1))
            for q in range(8):
                nc.vector.tensor_mul(out=z[tdx][q][:, bsl],
                                     in0=hbc[q][:, bsl],
                                     in1=xbc[tdx][:, bsl])

        def z_slice(j):
            for tdx in range(4):
                z_slice_t(j, tdx)

        # tile-0 z multiplies interleave with the later xbc copies on DVE:
        # the t0 batch must not queue behind copies it does not need
        z_slice_t(0, 0)
        xbc_mm(1)
        z_slice_t(0, 1)
        xbc_mm(2)
        z_slice_t(0, 2)
        xbc_mm(3)
        z_slice_t(0, 3)

        # ---- main matmuls + output assembly ----
        def accum2(tdx, rhs_sb, col0, ncols, bias_cols, bsl, psum_ap,
                   bias_last=False):
            # bias/lb2 FIRST normally (the group then closes on the last z
            # chunk); for b-tile 0 the bias goes LAST so the late-arriving
            # BB load cannot gate the group start
            nmm = (8 + (0 if skip_lb2 else 1)
                   + ((1 if skip_bb2 else 2) if bias_cols is not None else 0))
            idx = 0

            def bias_mm(first, last):
                nc.tensor.matmul(psum_ap,
                                 lhsT=hh_sb[64:128, bsl],
                                 rhs=BB_sb[:, bias_cols[0]:bias_cols[1]],
                                 start=first, stop=last)

            if bias_cols is not None and not bias_last:
                idx += 1
                bias_mm(True, False)
                if not skip_bb2:
                    idx += 1
                    nc.tensor.matmul(psum_ap, lhsT=ones_sb[:, bsl],
                                     rhs=BB2_sb[:, bias_cols[0]:bias_cols[1]],
                                     start=False, stop=False)
            if not skip_lb2:
                idx += 1
                nc.tensor.matmul(psum_ap,
                                 lhsT=xbc[tdx][0:16, bsl],
                                 rhs=rhs_sb[0:16, 8, col0:col0 + ncols],
                                 start=(idx == 1), stop=False)
            for q in range(8):
                idx += 1
                nc.tensor.matmul(psum_ap,
                                 lhsT=z[tdx][q][:, bsl],
                                 rhs=rhs_sb[:, q, col0:col0 + ncols],
                                 start=(idx == 1), stop=(idx == nmm))
            if bias_cols is not None and bias_last:
                if not skip_bb2:
                    idx += 1
                    nc.tensor.matmul(psum_ap, lhsT=ones_sb[:, bsl],
                                     rhs=BB2_sb[:, bias_cols[0]:bias_cols[1]],
                                     start=False, stop=False)
                idx += 1
                bias_mm(False, True)

        def emit_p01(j, out_t, o3, top, bsl, half=None):
            # r01k -> blk01: out[u, 32+3v+k], u<32, v<16
            # half=0/1 computes only u 0:16 / 16:32 (256 psum columns)
            for k in range(3):
                if half is None:
                    c0, cn, u0 = 0, 512, 0
                else:
                    c0, cn, u0 = 256 * half, 256, 16 * half
                p01 = main_psum.tile([P, 512], F32, name=f"p01_{k}", tag="mp")
                accum2(1 + k, R1_sb, c0, cn, None, bsl, p01[:, 0:cn])
                dst = top[:, u0:u0 + cn // 16, 32:80].rearrange(
                    "p u (v jj) -> p u v jj", jj=3)[:, :, :, k]
                src = p01[:, 0:cn].rearrange("p (u v) -> p u v", v=16)
                if k == 0:
                    nc.scalar.copy(dst, src)
                else:
                    nc.vector.tensor_copy(out=dst, in_=src)

        def emit_p00(j, out_t, o3, bsl, which, fine_tail, split_w=False):
            # r00 -> blk00 rows 0..15 (a) / 16..31 (b), plus the top write
            # for those rows once the p01 columns are in place
            col0, bias, r0, wa, wb = (
                (0, (0, 512), 0, 0, 1280) if which == 0
                else (512, (512, 1024), 16, 1280, 2560))
            if fine_tail:
                # the tail: one 256-column accumulation group (rows r0+8hh
                # .. +8) followed by its 640-column write; the very last
                # quarter splits once more so the final transfer is 320
                hh = fine_tail - 1
                p00 = main_psum.tile([P, 512], F32,
                                     name=f"p00{which}_{hh}", tag="mp")
                accum2(0, R0_sb, col0 + 256 * hh, 256,
                       (bias[0] + 256 * hh, bias[0] + 256 * hh + 256),
                       bsl, p00[:, 0:256])
                pv = p00[:, 0:256].rearrange("p (u v) -> p u v", v=32)
                rr = r0 + 8 * hh
                w0 = wa + 640 * hh
                if split_w:
                    nc.scalar.copy(o3[:, rr:rr + 4, 0:32], pv[:, 0:4])
                    nc.sync.dma_start(t["out"][bsl, w0:w0 + 320],
                                      out_t[:, w0:w0 + 320])
                    nc.scalar.copy(o3[:, rr + 4:rr + 8, 0:32], pv[:, 4:8])
                    nc.sync.dma_start(t["out"][bsl, w0 + 320:w0 + 640],
                                      out_t[:, w0 + 320:w0 + 640])
                else:
                    nc.scalar.copy(o3[:, rr:rr + 8, 0:32], pv)
                    nc.sync.dma_start(t["out"][bsl, w0:w0 + 640],
                                      out_t[:, w0:w0 + 640])
            else:
                p00 = main_psum.tile([P, 512], F32, name=f"p00{which}", tag="mp")
                accum2(0, R0_sb, col0, 512, bias, bsl, p00[:])
                pv = p00[:].rearrange("p (u v) -> p u v", v=32)
                nc.scalar.copy(o3[:, r0:r0 + 16, 0:32], pv)
                nc.sync.dma_start(t["out"][bsl, wa:wb], out_t[:, wa:wb])

        def emit_top(j, out_t, o3, top, bsl, fine_tail=False):
            if fine_tail:
                # last tile: p01 banks first, then p00 quarter-banks, each
                # followed by its 640-column write so the final writebacks
                # stream behind the closing matmuls
                emit_p01(j, out_t, o3, top, bsl)
                for which in (0, 1):
                    for hh in range(2):
                        emit_p00(j, out_t, o3, bsl, which, hh + 1,
                                 split_w=(which == 1 and hh == 1))
            else:
                emit_p00a_then_p01(j, out_t, o3, top, bsl)

        def emit_p00a_then_p01(j, out_t, o3, top, bsl):
            p00a = main_psum.tile([P, 512], F32, name="p00a", tag="mp")
            accum2(0, R0_sb, 0, 512, (0, 512), bsl, p00a[:])
            nc.scalar.copy(o3[:, 0:16, 0:32],
                           p00a[:].rearrange("p (u v) -> p u v", v=32))
            emit_p01(j, out_t, o3, top, bsl)
            nc.sync.dma_start(t["out"][bsl, 0:1280], out_t[:, 0:1280])
            p00b = main_psum.tile([P, 512], F32, name="p00b", tag="mp")
            accum2(0, R0_sb, 512, 512, (512, 1024), bsl, p00b[:])
            nc.scalar.copy(o3[:, 16:32, 0:32],
                           p00b[:].rearrange("p (u v) -> p u v", v=32))
            nc.sync.dma_start(t["out"][bsl, 1280:2560], out_t[:, 1280:2560])

        def emit_bot(j, out_t, o3, bot, bsl):
            # r10i -> blk10: out[32+3u+i, v], u<16, v<32
            # (before r11 so the bank order matches R-chunk arrival order)
            for i in range(3):
                p10 = main_psum.tile([P, 512], F32, name=f"p10_{i}", tag="mp")
                accum2(1 + i, R1_sb, 512, 512, None, bsl, p10[:])
                dst = bot[:, :, i, 0:32]                             # [128,16,32]
                src = p10[:].rearrange("p (u v) -> p u v", v=32)
                if i == 1:
                    nc.vector.tensor_copy(out=dst, in_=src)
                else:
                    # i==0,2 on ACT: GPSIMD cannot read PSUM
                    nc.scalar.copy(dst, src)

            # r11 -> blk11 diagonal-in-(i,j): out[32+3u+i, 32+3v+i]
            p11 = main_psum.tile([P, 512], F32, name="p11", tag="mp")
            accum2(0, R0_sb, 1024, 256, (1024, 1280), bsl, p11[:, 0:256])
            src11 = p11[:, 0:256].rearrange("p (u v) -> p u v", v=16)
            for i in range(3):
                dst = bot[:, :, i, 32:80].rearrange(
                    "p u (v jj) -> p u v jj", jj=3)[:, :, :, i]      # [128,16,16]
                if i == 1:
                    nc.vector.tensor_copy(out=dst, in_=src11)
                else:
                    # i==0,2 on ACT: GPSIMD cannot read PSUM
                    nc.scalar.copy(dst, src11)

            # bottom half in three chunks for tight writeback pipelining
            nc.sync.dma_start(t["out"][bsl, 2560:3840], out_t[:, 2560:3840])
            nc.sync.dma_start(t["out"][bsl, 3840:5120], out_t[:, 3840:5120])
            nc.sync.dma_start(t["out"][bsl, 5120:6400], out_t[:, 5120:6400])

        for j in range(NB):
            bsl = slice(P * j, P * (j + 1))
            out_t = opool.tile([P, 6400], F32, name="out_t", tag="out_t")
            o3 = out_t.rearrange("p (r c) -> p r c", c=80)          # [128,80,80]
            top = o3[:, 0:32, :]                                     # [128,32,80]
            bot = out_t[:, 2560:6400].rearrange(
                "p (u i c) -> p u i c", i=3, c=80)                   # [128,16,3,80]

            # blk11 off-diagonal zeros
            nc.gpsimd.memset(o3[:, 32:80, 32:80], 0.0)

            if j < NB - 1:
                emit_top(j, out_t, o3, top, bsl)
                # next tile's z slices build on DVE under this tile's
                # bottom banks (after the top copies, which gate the
                # first writeback)
                z_slice(j + 1)
                emit_bot(j, out_t, o3, bot, bsl)
            else:
                # last tile bottom-first: the tail then ends on the two
                # small top writes instead of the three bottom ones
                emit_bot(j, out_t, o3, bot, bsl)
                emit_top(j, out_t, o3, top, bsl, fine_tail=True)


def _prepare(inputs, mode):
    f32 = np.float32
    bf16 = ml_dtypes.bfloat16
    feat = np.ascontiguousarray(np.asarray(inputs["feat"], dtype=f32))
    node_emb = np.ascontiguousarray(np.asarray(inputs["node_emb"], dtype=f32))
    W0 = np.asarray(inputs["W0"], f32)
    W1 = np.asarray(inputs["W1"], f32)
    lw1 = np.asarray(inputs["lw1"], f32)
    lb1 = np.asarray(inputs["lb1"], f32)
    lw2 = np.asarray(inputs["lw2"], f32)
    lb2 = np.asarray(inputs["lb2"], f32)
    bw1 = np.asarray(inputs["bw1"], f32)
    bb1 = np.asarray(inputs["bb1"], f32)
    bw2 = np.asarray(inputs["bw2"], f32)
    bb2 = np.asarray(inputs["bb2"], f32)

    s16 = np.float32(1.0 / 16.0)
    sC = np.float32(C3 / 16.0)

    lw2p = np.concatenate([lw2, lb2[None]], axis=0)           # [65, 36864]
    M00 = lw2p[:, :16384].reshape(1040, 1024) * s16
    M11 = lw2p[:, 16384:20480].reshape(1040, 256) * sC
    M01 = lw2p[:, 20480:28672].reshape(1040, 512) * sC
    M10 = lw2p[:, 28672:36864].reshape(1040, 512) * sC
    R0 = np.ascontiguousarray(np.concatenate([M00, M11], axis=1)).astype(bf16)
    R1 = np.ascontiguousarray(np.concatenate([M01, M10], axis=1)).astype(bf16)
    BB = np.ascontiguousarray(
        np.concatenate([bw2[:, :1024] * s16, bw2[:, 1024:] * sC], axis=1)
    ).astype(bf16)                                            # [64, 1280]
    BB2 = np.ascontiguousarray(np.concatenate(
        [bb2[None, :1024] * s16, bb2[None, 1024:] * sC], axis=1)).astype(bf16)

    W0s = W0 * np.float32(1.0 / np.sqrt(128.0))
    W1s = W1 * np.float32(1.0 / 8.0)

    # column-replicated pre-matmul weights: the PE emits partition-replicated
    # activations directly (see module docstring)
    # Gsel[c, j] = (j//16 == c): the one-hot post-SiLU replication matmul
    Gsel = np.zeros((P, 1024), bf16)
    Gsel[0:64] = (np.arange(1024) // 16 ==
                  np.arange(64)[:, None]).astype(bf16)
    # stacked [lw1 | bw1] -> one matmul, one SiLU for h and hb
    LWB1 = np.concatenate([lw1, bw1], axis=1).astype(bf16)    # [128, 128]
    lbb1 = np.concatenate([lb1, bb1])[:, None].astype(bf16)   # [128, 1]
    W0R = np.tile(W0s, (1, 8)).astype(bf16)                   # [128, 128]
    W1R = np.zeros((P, P), bf16)
    W1R[0:64] = np.tile(W1s, (1, 8)).astype(bf16)             # [64, 128] + pad


    skip_lb2 = not bool(np.any(lb2))
    skip_bb2 = not bool(np.any(bb2))

    in_maps = []
    for i in range(N_CORES):
        sl = slice(i * BC, (i + 1) * BC)
        fs = feat[sl]
        embT = node_emb[sl].T.astype(bf16)                    # [128, BC]
        featsT = fs[:, :128].T.astype(bf16)                   # [128, BC]
        featv = np.stack(
            [fs[:, 128 + k::3].T.astype(bf16) for k in range(3)], axis=1
        )                                                     # [64, 3, BC]
        blob16 = np.ascontiguousarray(np.concatenate(
            [embT, LWB1, lbb1, featsT, W0R,
             Gsel, W1R], axis=1))                             # [128, 2433]
        m = {
            "blob16": blob16,
            "featv": np.ascontiguousarray(featv),
            "R0": R0, "R1": R1, "BB": BB,
        }
        if not skip_bb2:
            m["BB2"] = BB2
        in_maps.append(m)
    return in_maps, skip_lb2, skip_bb2


def run(inputs, mode=None, trace=False):
    """Build (cached), run on 8 cores, gather. Returns (out, results)."""
    mode = mode or MM_MODE
    in_maps, skip_lb2, skip_bb2 = _prepare(inputs, mode)
    key = (mode, skip_lb2, skip_bb2)
    if key not in _CACHE:
        _CACHE[key] = _build_program(mode, skip_lb2, skip_bb2)
    nc = _CACHE[key]

    from concourse.bass_utils import run_bass_kernel_spmd
    res = run_bass_kernel_spmd(nc, in_maps, list(range(N_CORES)), trace=trace)
    out = np.concatenate(
        [res.results[i]["out"].reshape(BC, 80, 80) for i in range(N_CORES)],
        axis=0)
    return out.astype(np.float32), res


def kernel(**inputs):
    out, _ = run(inputs)
    return out
